# revision 9
# baseline (speedup 1.0000x reference)
"""GINE GNN (2-layer) + mean-pool kernel for 8 trn2 NeuronCores.

Sharding: nodes/edges by destination across 8 cores (graph parallel).
Edges sorted by (src-half, dst), packed into 128-node blocks with a fixed
tile count (SPMD-uniform). Per block: dma_gather x[src] (int16 half
tables, 4 SWDGE queues), block-diag fp32r matmul for edge_attr @ We (+bias
via ones-row), identity-matmul add of gathered rows, ACT relu, DVE
is_equal one-hot, per-tile scatter matmuls accumulating transposed
aggregates in PSUM, DVE add into an SBUF-resident transposed node table.
MLP: weights-stationary transposed matmuls over node spans; LayerNorm in
node-major layout between PE transposes. Cross-core: AllGather of updated
node features between layers; AllReduce of per-graph pool sums.
"""
import sys
sys.path.insert(0, "/opt/trn_rl_repo")
sys.path.insert(0, "/root/problem")
import numpy as np
import concourse.bass as bass
import concourse.bacc as bacc
import concourse.mybir as mybir
import concourse.tile as tile
from concourse.bass_utils import run_bass_kernel_spmd

F32 = mybir.dt.float32
F32R = mybir.dt.float32r
I16 = mybir.dt.int16
AF = mybir.ActivationFunctionType
ALU = mybir.AluOpType

N_CORES = 8
N_NODES = 50000
N_GRAPHS = 1000
D0, EDIM, H, OUT = 78, 6, 128, 128
LN_EPS = 1e-5

NPAD = 50176
SHARD = NPAD // N_CORES      # 6272
HALF = NPAD // 2             # 25088
NBLK = SHARD // 128          # 49
GQ = 1024
SP = 896                     # MLP span (7 tiles), 7 spans per shard
NSP = SHARD // SP

_cache = {}


def _pack_idx16(idx):
    n = len(idx)
    return np.tile(np.asarray(idx, np.int16).reshape(n // 16, 16).T, (8, 1))


def _slices(total, step):
    return [slice(i, min(i + step, total)) for i in range(0, total, step)]


def _prep(x, edge_index, edge_attr, batch, params):
    src = np.asarray(edge_index[0], np.int64)
    dst = np.asarray(edge_index[1], np.int64)
    ea = np.asarray(edge_attr, np.float32)
    batch = np.asarray(batch, np.int64)

    xpad = np.zeros((NPAD, H), np.float32)
    xpad[:N_NODES, :D0] = np.asarray(x, np.float32)

    core_of = dst // SHARD
    percore = []
    max_ta = max_tb = 0
    for c in range(N_CORES):
        m = core_of == c
        s, d, e = src[m], dst[m] - c * SHARD, ea[m]
        half = (s >= HALF).astype(np.int64)
        blk = d // 128
        order = np.lexsort((s, d, half))
        s, d, e, half, blk = s[order], d[order], e[order], half[order], blk[order]
        lists = {}
        for b in range(NBLK):
            bm = blk == b
            for g in (0, 1):
                sel = bm & (half == g)
                lists[(b, g)] = (s[sel], d[sel], e[sel])
                t = (int(sel.sum()) + 127) // 128
                if g == 0:
                    max_ta = max(max_ta, t)
                else:
                    max_tb = max(max_tb, t)
        percore.append(lists)

    TA, TB = int(max_ta), int(max_tb)
    T = TA + TB
    KBD = 6 * T + 1
    assert KBD <= 128, f"block-diag K too large: {KBD}"

    cnt = np.zeros(GQ, np.float32)
    np.add.at(cnt, batch, 1.0)
    recip = (1.0 / np.maximum(cnt, 1.0)).astype(np.float32)

    batch_pad = np.full(NPAD, GQ - 1, np.int64)
    batch_pad[:N_NODES] = batch

    L0, L1 = params['layers'][0], params['layers'][1]
    We1 = np.zeros((EDIM, H), np.float32)
    We1[:, :D0] = np.asarray(L0['We'], np.float32)
    be1 = np.zeros(H, np.float32)
    be1[:D0] = np.asarray(L0['be'], np.float32)
    We2 = np.asarray(L1['We'], np.float32)
    be2 = np.asarray(L1['be'], np.float32)

    def blockdiag(We, be):
        bd = np.zeros((KBD, T * 128), np.float32)
        for t in range(T):
            bd[6 * t:6 * t + 6, 128 * t:128 * (t + 1)] = We
            bd[6 * T, 128 * t:128 * (t + 1)] = be
        return bd

    W1l1 = np.zeros((H, H), np.float32)
    W1l1[:D0] = np.asarray(L0['W1'], np.float32)
    W1l1[127, :] += np.asarray(L0['b1'], np.float32)  # ones-row bias fold

    iota128 = np.tile(np.arange(128, dtype=np.float32)[None, :], (128, 1))

    meta = dict(TA=TA, TB=TB, T=T, KBD=KBD)
    in_maps = []
    for c in range(N_CORES):
        lists = percore[c]
        NIA, NIB = TA * 128, TB * 128
        dstm = np.full((NBLK, 128, T), -1.0, np.float32)
        eapk = np.zeros((NBLK, KBD, 128), np.float32)
        eapk[:, 6 * T, :] = 1.0
        giA = np.zeros((NBLK, 128, NIA // 16), np.int16)
        giB = np.zeros((NBLK, 128, NIB // 16), np.int16)
        for b in range(NBLK):
            for g, gi_arr, nig, toff in ((0, giA, NIA, 0), (1, giB, NIB, TA)):
                s, d, e = lists[(b, g)]
                n = len(s)
                gi = np.zeros(nig, np.int64)
                gi[:n] = s - g * HALF
                gi_arr[b] = _pack_idx16(gi)
                if n:
                    i = np.arange(n)
                    t_ = toff + i // 128
                    p_ = i % 128
                    dstm[b, p_, t_] = (d - b * 128).astype(np.float32)
                    for j in range(EDIM):
                        eapk[b, 6 * t_ + j, p_] = e[:, j]

        xT_l1 = xpad.T[:, c * SHARD:(c + 1) * SHARD].copy()
        xT_l1[127, :] = 1.0

        bl = batch_pad[c * SHARD:(c + 1) * SHARD].astype(np.float32)

        im = dict(
            x_tbl=xpad,
            xT_l1=xT_l1,
            giA=giA, giB=giB, dstm=dstm, eapk=eapk,
            BD1=blockdiag(We1, be1), BD2=blockdiag(We2, be2),
            iota128=iota128,
            batchloc=bl.reshape(NBLK, 128).T.copy(),
            iotaG=np.tile(np.arange(GQ, dtype=np.float32)[None, :], (128, 1)),
            W1l1=W1l1,
            W2l1=np.asarray(L0['W2'], np.float32),
            W1l2=np.asarray(L1['W1'], np.float32),
            W2l2=np.asarray(L1['W2'], np.float32),
            Wp=np.asarray(params['Wp'], np.float32),
            b2l1=np.tile(np.asarray(L0['b2'], np.float32).reshape(1, H), (128, 1)),
            b1l2=np.tile(np.asarray(L1['b1'], np.float32).reshape(1, H), (128, 1)),
            b2l2=np.tile(np.asarray(L1['b2'], np.float32).reshape(1, H), (128, 1)),
            g1l1=np.tile(np.asarray(L0['g1'], np.float32).reshape(1, H), (128, 1)),
            bt1l1=np.tile(np.asarray(L0['bt1'], np.float32).reshape(1, H), (128, 1)),
            g2l1=np.tile(np.asarray(L0['g2'], np.float32).reshape(1, H), (128, 1)),
            bt2l1=np.tile(np.asarray(L0['bt2'], np.float32).reshape(1, H), (128, 1)),
            g1l2=np.tile(np.asarray(L1['g1'], np.float32).reshape(1, H), (128, 1)),
            bt1l2=np.tile(np.asarray(L1['bt1'], np.float32).reshape(1, H), (128, 1)),
            g2l2=np.tile(np.asarray(L1['g2'], np.float32).reshape(1, H), (128, 1)),
            bt2l2=np.tile(np.asarray(L1['bt2'], np.float32).reshape(1, H), (128, 1)),
            bp=np.asarray(params['bp'], np.float32).reshape(1, OUT),
            recip=np.tile(recip.reshape(1, GQ), (128, 1)),
            ones1=np.ones((1, 128), np.float32),
            ident=np.eye(128, dtype=np.float32),
        )
        in_maps.append(im)
    return meta, in_maps


def _build(meta):
    TA, TB, T, KBD = meta['TA'], meta['TB'], meta['T'], meta['KBD']
    NIA, NIB = TA * 128, TB * 128
    NE = T * 128

    nc = bacc.Bacc('TRN2', target_bir_lowering=False, debug=False,
                   num_devices=N_CORES, num_swdge_queues=4)

    def din(name, shape, dt=F32R):
        return nc.dram_tensor(name, shape, dt, kind="ExternalInput")

    x_tbl = din('x_tbl', [NPAD, H])
    xT_l1 = din('xT_l1', [128, SHARD])
    giA = din('giA', [NBLK, 128, NIA // 16], I16)
    giB = din('giB', [NBLK, 128, NIB // 16], I16)
    dstm = din('dstm', [NBLK, 128, T], F32)
    eapk = din('eapk', [NBLK, KBD, 128])
    BDs = {0: din('BD1', [KBD, NE]), 1: din('BD2', [KBD, NE])}
    iota128 = din('iota128', [128, 128], F32)
    batchloc = din('batchloc', [128, NBLK], F32)
    iotaG = din('iotaG', [128, GQ], F32)
    Ws = {n: din(n, [H, H]) for n in ('W1l1', 'W2l1', 'W1l2', 'W2l2', 'Wp')}
    vec = {n: din(n, [128, H], F32) for n in
           ('b2l1', 'b1l2', 'b2l2', 'g1l1', 'bt1l1', 'g2l1', 'bt2l1',
            'g1l2', 'bt1l2', 'g2l2', 'bt2l2')}
    vec['bp'] = din('bp', [1, H], F32R)
    recip = din('recip', [128, GQ], F32)
    ones1 = din('ones1', [1, 128])
    ident = din('ident', [128, 128])

    out_dram = nc.dram_tensor('out', [GQ, OUT], F32, kind="ExternalOutput")
    x2_loc = nc.dram_tensor('x2_loc', [SHARD, H], F32R)
    x2_full = nc.dram_tensor('x2_full', [NPAD, H], F32R)
    pool_in = nc.dram_tensor('pool_in', [128, GQ], F32)
    pool_out = nc.dram_tensor('pool_out', [128, GQ], F32)

    with tile.TileContext(nc) as tc:
        with tc.tile_pool(name="const", bufs=1) as cpool, \
             tc.tile_pool(name="big", bufs=1) as bpool, \
             tc.tile_pool(name="work", bufs=2) as wpool, \
             tc.tile_pool(name="small", bufs=2) as spool, \
             tc.tile_pool(name="psA", bufs=1, space="PSUM") as psA, \
             tc.tile_pool(name="psB", bufs=1, space="PSUM") as psB:

            eps_t = cpool.tile([128, 1], F32, tag='c_eps')
            nc.vector.memset(eps_t[:], LN_EPS)
            iota_t = cpool.tile([128, 128], F32, tag='c_iota')
            nc.sync.dma_start(out=iota_t[:], in_=iota128[:])
            ident_t = cpool.tile([128, 128], F32R, tag='c_ident')
            nc.sync.dma_start(out=ident_t[:], in_=ident[:])
            ones1_t = cpool.tile([1, 128], F32R, tag='c_ones1')
            nc.sync.dma_start(out=ones1_t[:], in_=ones1[:])
            W_t = {}
            for nm in Ws:
                t_ = cpool.tile([H, H], F32R, tag=f'c_{nm}')
                nc.sync.dma_start(out=t_[:], in_=Ws[nm][:])
                W_t[nm] = t_
            v_t = {}
            for nm in vec:
                shp = [1, H] if nm == 'bp' else [128, H]
                t_ = cpool.tile(shp, F32R if nm == 'bp' else F32, tag=f'c_{nm}')
                nc.sync.dma_start(out=t_[:], in_=vec[nm][:])
                v_t[nm] = t_
            recip_t = cpool.tile([128, GQ], F32, tag='c_recip')
            nc.sync.dma_start(out=recip_t[:], in_=recip[:])
            batchloc_t = cpool.tile([128, NBLK], F32, tag='c_batchloc')
            nc.sync.dma_start(out=batchloc_t[:], in_=batchloc[:])
            iotaG_t = cpool.tile([128, GQ], F32, tag='c_iotaG')
            nc.sync.dma_start(out=iotaG_t[:], in_=iotaG[:])

            hT = bpool.tile([128, SHARD], F32R, tag='b_hT')
            x2T = bpool.tile([128, SHARD], F32R, tag='b_x2T')
            a1T = bpool.tile([128, SP], F32R, tag='b_a1T')
            bd_t = bpool.tile([KBD, NE], F32R, tag='b_BD')

            def edge_phase(layer):
                tbl = x_tbl if layer == 0 else x2_full
                nc.sync.dma_start(out=bd_t[:], in_=BDs[layer][:])
                gih = (giA, giB)
                nis = (NIA, NIB)
                tas = (TA, TB)
                for b in range(NBLK):
                    xg = wpool.tile([128, T, H], F32R, tag='w_xg')
                    qn = 0
                    for g in (0, 1):
                        it = spool.tile([128, nis[g] // 16], I16, tag=f'w_gi{g}')
                        nc.sync.dma_start(out=it[:], in_=gih[g][b])
                        toff = 0 if g == 0 else TA
                        # split tiles so each gather has <= 1024 descriptors
                        tsp = [t for t in range(0, tas[g], 8)]
                        for t0 in tsp:
                            ntg = min(8, tas[g] - t0)
                            nc.gpsimd.dma_gather(
                                out_ap=xg[:, toff + t0:toff + t0 + ntg, :],
                                in_ap=tbl[g * HALF:(g + 1) * HALF, :],
                                idxs_ap=it[:, (t0 * 128) // 16:
                                           ((t0 + ntg) * 128) // 16],
                                num_idxs=ntg * 128, num_idxs_reg=ntg * 128,
                                elem_size=H, queue_num=qn % 4,
                            )
                            qn += 1
                    ea_t = spool.tile([KBD, 128], F32R, tag='w_ea')
                    nc.sync.dma_start(out=ea_t[:], in_=eapk[b])
                    msg_ps = psA.tile([128, NE], F32, tag='P_big')
                    for sl in _slices(NE, 512):
                        nc.tensor.matmul(msg_ps[:, sl], ea_t[:], bd_t[:, sl],
                                         start=True, stop=False)
                    xgf = xg[:].rearrange("p t h -> p (t h)")
                    sls = _slices(NE, 512)
                    for i, sl in enumerate(sls):
                        nc.tensor.matmul(msg_ps[:, sl], ident_t[:], xgf[:, sl],
                                         start=False, stop=(i == len(sls) - 1))
                    msg = wpool.tile([128, T, H], F32R, tag='w_msg')
                    nc.scalar.activation(
                        msg[:].rearrange("p t h -> p (t h)"), msg_ps[:],
                        AF.Relu)
                    dt_ = spool.tile([128, T], F32, tag='w_dstm')
                    nc.sync.dma_start(out=dt_[:], in_=dstm[b])
                    oh = wpool.tile([128, T, 128], F32R, tag='w_oh')
                    nc.vector.tensor_tensor(
                        out=oh[:],
                        in0=dt_[:].rearrange("p (t o) -> p t o", o=1)
                                  .to_broadcast([128, T, 128]),
                        in1=iota_t[:].rearrange("p (o n) -> p o n", o=1)
                                     .to_broadcast([128, T, 128]),
                        op=ALU.is_equal)
                    agg_ps = psB.tile([128, 128], F32, tag='P_agg')
                    for t in range(T):
                        nc.tensor.matmul(agg_ps[:], msg[:, t, :], oh[:, t, :],
                                         start=(t == 0), stop=(t == T - 1))
                    blk = slice(128 * b, 128 * (b + 1))
                    if layer == 0:
                        xt_t = spool.tile([128, 128], F32R, tag='w_xt')
                        nc.sync.dma_start(out=xt_t[:], in_=xT_l1[:, blk])
                        nc.vector.tensor_tensor(out=hT[:, blk], in0=xt_t[:],
                                                in1=agg_ps[:], op=ALU.add)
                    else:
                        nc.vector.tensor_tensor(out=hT[:, blk], in0=x2T[:, blk],
                                                in1=agg_ps[:], op=ALU.add)

            def lnblock(zT_ps, gname, btname, bias_row, dstT, rows_dram,
                        span_off):
                for q in range(SP // 128):
                    qs = slice(128 * q, 128 * (q + 1))
                    zTs = spool.tile([128, 128], F32R, tag='m_zT')
                    nc.scalar.activation(zTs[:], zT_ps[:, qs], AF.Copy)
                    zn_ps = psB.tile([128, 128], F32R, tag='P_tr')
                    nc.tensor.matmul(zn_ps[:], zTs[:], ident_t[:],
                                     is_transpose=True)
                    z = spool.tile([128, 128], F32, tag='m_z')
                    musum = spool.tile([128, 1], F32, tag='m_mu')
                    nc.scalar.activation(z[:], zn_ps[:].bitcast(F32), AF.Copy,
                                         accum_out=musum[:])
                    if bias_row is not None:
                        nc.vector.tensor_tensor(
                            out=z[:], in0=z[:],
                            in1=bias_row[:],
                            op=ALU.add)
                        nc.vector.tensor_reduce(
                            out=musum[:], in_=z[:],
                            axis=mybir.AxisListType.X, op=ALU.add)
                    negmu = spool.tile([128, 1], F32, tag='m_negmu')
                    nc.vector.tensor_scalar_mul(negmu[:], musum[:], -1.0 / H)
                    sq = spool.tile([128, 128], F32, tag='m_sq')
                    varsum = spool.tile([128, 1], F32, tag='m_vs')
                    nc.scalar.activation(sq[:], z[:], AF.Square,
                                         bias=negmu[:], accum_out=varsum[:])
                    std = spool.tile([128, 1], F32, tag='m_std')
                    nc.scalar.activation(std[:], varsum[:], AF.Sqrt,
                                         scale=1.0 / H, bias=eps_t[:])
                    rstd = spool.tile([128, 1], F32, tag='m_rstd')
                    nc.vector.reciprocal(rstd[:], std[:])
                    ln = spool.tile([128, 128], F32, tag='m_ln')
                    nc.vector.tensor_scalar(ln[:], z[:], negmu[:], rstd[:],
                                            ALU.add, ALU.mult)
                    aff = spool.tile([128, 128], F32, tag='m_aff')
                    nc.vector.tensor_tensor(
                        out=aff[:], in0=ln[:],
                        in1=v_t[gname][:], op=ALU.mult)
                    nc.vector.tensor_tensor(
                        out=aff[:], in0=aff[:],
                        in1=v_t[btname][:], op=ALU.add)
                    relu = spool.tile([128, 128], F32R, tag='m_relu')
                    nc.scalar.activation(relu[:], aff[:], AF.Relu)
                    if rows_dram is not None:
                        nc.sync.dma_start(
                            out=rows_dram[span_off + 128 * q:
                                          span_off + 128 * (q + 1), :],
                            in_=relu[:])
                    if dstT is not None:
                        rT_ps = psB.tile([128, 128], F32R, tag='P_tr')
                        nc.tensor.matmul(rT_ps[:], relu[:], ident_t[:],
                                         is_transpose=True)
                        doff = span_off if dstT is a1T else span_off
                        nc.scalar.activation(
                            dstT[:, doff + 128 * q:doff + 128 * (q + 1)],
                            rT_ps[:].bitcast(F32), AF.Copy)

            def mlp_phase(layer):
                W1 = W_t['W1l1' if layer == 0 else 'W1l2']
                W2 = W_t['W2l1' if layer == 0 else 'W2l2']
                sfx = 'l1' if layer == 0 else 'l2'
                for sp in range(NSP):
                    base = SP * sp
                    z1_ps = psA.tile([128, SP], F32, tag='P_big')
                    for sl in _slices(SP, 512):
                        nc.tensor.matmul(
                            z1_ps[:, sl], W1[:],
                            hT[:, base + sl.start:base + sl.stop],
                            start=True, stop=True)
                    lnblock(z1_ps, f'g1{sfx}', f'bt1{sfx}',
                            v_t[f'b1{sfx}'] if layer == 1 else None,
                            a1T, None, 0)
                    z2_ps = psA.tile([128, SP], F32, tag='P_big')
                    for sl in _slices(SP, 512):
                        nc.tensor.matmul(z2_ps[:, sl], W2[:], a1T[:, sl],
                                         start=True, stop=True)
                    if layer == 0:
                        lnblock(z2_ps, f'g2{sfx}', f'bt2{sfx}',
                                v_t[f'b2{sfx}'], x2T, x2_loc, base)
                    else:
                        lnblock(z2_ps, f'g2{sfx}', f'bt2{sfx}',
                                v_t[f'b2{sfx}'], hT, None, base)

            edge_phase(0)
            mlp_phase(0)
            nc.gpsimd.collective_compute(
                "AllGather", ALU.bypass,
                ins=[x2_loc[:]], outs=[x2_full[:]],
                replica_groups=[list(range(N_CORES))])
            edge_phase(1)
            mlp_phase(1)  # writes final features (transposed) into hT

            poolT_ps = psA.tile([128, GQ], F32, tag='P_big')
            for b in range(NBLK):
                ohg = wpool.tile([128, GQ], F32R, tag='w_oh')
                nc.vector.tensor_tensor(
                    out=ohg[:],
                    in0=batchloc_t[:, b:b + 1].to_broadcast([128, GQ]),
                    in1=iotaG_t[:], op=ALU.is_equal)
                xr_ps = psB.tile([128, 128], F32R, tag='P_tr')
                nc.tensor.matmul(xr_ps[:], hT[:, 128 * b:128 * (b + 1)],
                                 ident_t[:], is_transpose=True)
                xr = spool.tile([128, 128], F32R, tag='w_xt')
                nc.scalar.activation(xr[:], xr_ps[:].bitcast(F32), AF.Copy)
                for s, sl in enumerate(_slices(GQ, 512)):
                    nc.tensor.matmul(poolT_ps[:, sl], xr[:], ohg[:, sl],
                                     start=(b == 0), stop=(b == NBLK - 1))
            poolT = wpool.tile([128, GQ], F32, tag='w_msg')
            nc.scalar.activation(poolT[:], poolT_ps[:], AF.Copy)
            nc.sync.dma_start(out=pool_in[:], in_=poolT[:])
            nc.gpsimd.collective_compute(
                "AllReduce", ALU.add,
                ins=[pool_in[:]], outs=[pool_out[:]],
                replica_groups=[list(range(N_CORES))])
            poolF = wpool.tile([128, GQ], F32R, tag='w_oh')
            nc.sync.dma_start(out=poolF[:].bitcast(F32), in_=pool_out[:])
            pooled = wpool.tile([128, GQ], F32R, tag='w_xg')
            nc.vector.tensor_tensor(
                out=pooled[:], in0=poolF[:],
                in1=recip_t[:], op=ALU.mult)
            for q in range(GQ // 128):
                qs = slice(128 * q, 128 * (q + 1))
                o_ps = psB.tile([128, OUT], F32, tag='P_agg')
                nc.tensor.matmul(o_ps[:], ones1_t[:],
                                 v_t['bp'][:],
                                 start=True, stop=False)
                nc.tensor.matmul(o_ps[:], pooled[:, qs], W_t['Wp'][:],
                                 start=False, stop=True)
                o_sb = spool.tile([128, OUT], F32, tag='m_z')
                nc.scalar.activation(o_sb[:], o_ps[:], AF.Copy)
                nc.sync.dma_start(out=out_dram[qs, :], in_=o_sb[:])

    nc.compile()
    return nc


def kernel(x, edge_index, edge_attr, batch, params):
    meta, in_maps = _prep(x, edge_index, edge_attr, batch, params)
    key = (meta['TA'], meta['TB'])
    if key not in _cache:
        _cache[key] = _build(meta)
    res = run_bass_kernel_spmd(_cache[key], in_maps, list(range(N_CORES)))
    return np.asarray(res.results[0]['out'])[:N_GRAPHS].astype(np.float32)
